# revision 9
# baseline (speedup 1.0000x reference)
"""Trainium2 Bass kernel for a contrastive hinge loss.

Problem (B=32 splits, L=1024 candidates/split, P=8 positives/split, D=256):
    e = l2norm(sent), q = l2norm(query)
    sim[b,l] = e[b,l] . q[b]
    loss = sum_{b, p in pos_b, j in neg_b} relu(sim[b,j] - sim[b,p] + margin) / total

Strategy (data-parallel over B across 8 cores, 4 splits per core), v5:
  bf16 inputs (tolerance 2e-2; bf16 keeps the loss within ~2e-5) halve the
  HBM traffic, and x is host-transposed to d-major [d, l] so the dot
  reductions run on the PE.  All dot/ssq results live in ONE [8, 512] psum
  row tile each, partition (4c + b) = l-chunk c of split b, so the whole
  normalization + hinge tail is a single chain:
    prd[4c+b, :] = sum_d q[b,d] x[b,d,l]     (PE, q as 8-col weights)
    prq[4c+b, :] = sum_d x[b,d,l]^2          (squares on DVE for 2 splits and
                                              ACT for 2, PE ones-fold)
    sim = prd * rsqrt(prq)                   (ACT sqrt, DVE approx-recip, mult)
    pss[64, 512] = Ind^T @ sim               (one PE matmul: broadcast to all
                                              (c, b, j) partitions)
    G2[(c,b,j)] = sum_l relu(pss + (margin - s_bj))   (ONE ACT Relu with
                                              per-partition bias + accumulator)
  G[b,j] = G2[0,b,j] + G2[1,b,j] summed on the host.
  s_vec[b,j] comes from host-gathered positive embeddings (aux), computed on
  partitions (c,b,j) directly: STT dot + ACT square-accum + sqrt/recip/mult.
  DMA: the ACT HWDGE ring is ~3-4x faster than the SP ring, so x rides ACT
  (splits 0+1 packed as one tensor for 8 KB descriptor lines, split 3 second)
  plus SWDGE (split 2); wts/aux go on the idle SP ring; out returns on ACT.
  Host finish: dedup positives + subtract pos-pos hinge terms, divide by total.
"""

import numpy as np

B, L, P, D = 32, 1024, 8, 256
NCORES = 8
BL = B // NCORES          # 4 splits per core
H = 2                     # d-halves (256 = 2*128)
C = 2                     # l-chunks (1024 = 2*512 psum-bank limit)
CH = 512
MARGIN = 0.01

# wts column layout (all bf16):
#   dot block for (b,h): 12 cols at WD + 12*(2b+h); q sits at local col 4+b,
#     so slice [4-4c : 12-4c] puts q at col (4c+b) of an 8-wide lhsT.
#   fold block for b: 12 cols at WF + 12*b; ones at local col 4+b, same trick.
#   ind block: 64 cols at WI; wts[4c+b, WI + 32c + 8b + j] = 1.
WD = 0
WF = WD + 12 * BL * H     # 96
WI = WF + 12 * BL         # 144
WTOT = WI + 2 * BL * P    # 208

_CACHED = {}


def _build_nc():
    import concourse.bass as bass
    import concourse.mybir as mybir
    import concourse.tile as tile
    from concourse import bacc

    f32 = mybir.dt.float32
    b16 = mybir.dt.bfloat16
    Alu = mybir.AluOpType
    Act = mybir.ActivationFunctionType

    nc = bacc.Bacc("TRN2")
    # x[b][p, h*1024 + l] = sent_T[b, 128h + p, l]  (host-transposed, bf16).
    # Splits 0+1 ride one tensor so each partition line is 8 KB contiguous --
    # the HWDGE rings are descriptor-rate-limited, so bigger lines = more B/s.
    x0 = nc.dram_tensor("x0", [128, H * L], b16, kind="ExternalInput")
    xpr = nc.dram_tensor("xpr", [128, 2 * H * L], b16, kind="ExternalInput")
    x3a = nc.dram_tensor("x3a", [128, L], b16, kind="ExternalInput")
    x3b = nc.dram_tensor("x3b", [128, L], b16, kind="ExternalInput")
    wts = nc.dram_tensor("wts", [128, WTOT], b16, kind="ExternalInput")
    # aux[32c + 8b + j, 0:256] = sent[b, pos_idx[b,j], :]; [., 256:512] = qhat[b]
    aux = nc.dram_tensor("aux", [2 * BL * P, 2 * D], b16, kind="ExternalInput")
    # out[32c + 8b + j] = (G2[c,b,j], s_vec[b,j])
    out = nc.dram_tensor("out", [2 * BL * P, 2], f32, kind="ExternalOutput")

    NP = 2 * BL * P       # 64 (c,b,j) partitions

    with tile.TileContext(nc) as tc:
        with (
            tc.tile_pool(name="singles", bufs=1) as singles,
            tc.tile_pool(name="xpool", bufs=4) as xpool,
            tc.tile_pool(name="sqpool", bufs=8) as sqpool,
            tc.tile_pool(name="pp", bufs=1, space="PSUM") as pp,
        ):
            # ---- x loads: split 0 rides alone for the earliest start,
            # splits 1+2 packed (8 KB lines), split 3 halved over SP+SWDGE ----
            x0_sb = xpool.tile([128, H * L], b16, tag="x0")
            nc.scalar.dma_start(out=x0_sb[:, :], in_=x0[:, :])
            xpr_sb = xpool.tile([128, 2 * H * L], b16, tag="xpr")
            nc.scalar.dma_start(out=xpr_sb[:, :], in_=xpr[:, :])
            wts_sb = singles.tile([128, WTOT], b16)
            nc.sync.dma_start(out=wts_sb[:, :], in_=wts[:, :])
            aux_sb = singles.tile([NP, 2 * D], b16)
            nc.sync.dma_start(out=aux_sb[:, :], in_=aux[:, :])
            x3a_sb = xpool.tile([128, L], b16, tag="x3a")
            nc.sync.dma_start(out=x3a_sb[:, :], in_=x3a[:, :])
            x3b_sb = xpool.tile([128, L], b16, tag="x3b")
            nc.gpsimd.dma_start(out=x3b_sb[:, :], in_=x3b[:, :])
            # per (b, h): the [128, L] block of x for that split/d-half
            xh = [[x0_sb[:, 0:L], x0_sb[:, L:2 * L]],
                  [xpr_sb[:, 0:L], xpr_sb[:, L:2 * L]],
                  [xpr_sb[:, 2 * L:3 * L], xpr_sb[:, 3 * L:4 * L]],
                  [x3a_sb, x3b_sb]]

            # ---- ACT table warm (sqrt_and_others: Sqrt/Square/Relu/Copy) ----
            warm = singles.tile([1, 1], f32)
            nc.vector.memset(warm[:, :], 1.0)
            nc.scalar.activation(out=warm[0:1, :], in_=warm[0:1, :],
                                 func=Act.Sqrt)

            # ---- PE warm-up: keep the HAM busy through the DMA window so
            # real matmuls run at 2.4 GHz instead of 1.2.
            junkw = singles.tile([128, 516], b16)
            nc.vector.memset(junkw[:, :], 0.125)
            pjunk = pp.tile([4, CH], f32, name="pjunk", tag="pjunk")
            for w in range(10):
                nc.tensor.matmul(pjunk[:, :], lhsT=junkw[:, 0:4],
                                 rhs=junkw[:, 4:516], start=True, stop=True)

            # ---- heavy pass: dot rows prd, ssq rows prq at (4c+b) ----
            prd = pp.tile([2 * BL, CH], f32, name="prd", tag="prd")
            prq = pp.tile([2 * BL, CH], f32, name="prq", tag="prq")
            pss = pp.tile([NP, CH], f32, name="pss", tag="pss")

            nd = 0
            nq = 0
            NTOT = BL * H * C     # 16 MMs in each of the two groups

            # squares run per (split, d-half) so the last split's square is
            # short; ACT takes two halves so DVE's chain stays under ~4.5us
            SQ_ACT = {(1, 1), (2, 1)}
            for b in [0, 3, 1, 2]:
                for h in range(H):
                    xbh = xh[b][h]
                    blk = WD + 12 * (2 * b + h)
                    for c in range(C):
                        nc.tensor.matmul(
                            prd[:, :],
                            lhsT=wts_sb[:, blk + 4 - 4 * c:blk + 12 - 4 * c],
                            rhs=xbh[:, c * CH:c * CH + CH],
                            start=(nd == 0),
                            stop=(nd == NTOT - 1),
                        )
                        nd += 1
                    sq = sqpool.tile([128, L], b16, tag="sq")
                    if (b, h) in SQ_ACT:
                        nc.scalar.activation(out=sq[:, :], in_=xbh[:, :],
                                             func=Act.Square)
                    else:
                        nc.vector.tensor_mul(out=sq[:, :], in0=xbh[:, :],
                                             in1=xbh[:, :])
                    fblk = WF + 12 * b
                    for c in range(C):
                        nc.tensor.matmul(
                            prq[:, :],
                            lhsT=wts_sb[:, fblk + 4 - 4 * c:fblk + 12 - 4 * c],
                            rhs=sq[:, c * CH:c * CH + CH],
                            start=(nq == 0),
                            stop=(nq == NTOT - 1),
                        )
                        nq += 1

            # ---- s_vec on (c,b,j) partitions (emitted late: only needed as
            # the relu bias; aux also arrives late on the SP ring) ----
            dp = singles.tile([NP, 1], f32)
            sp = singles.tile([NP, 1], f32)
            junka = singles.tile([NP, D], b16)
            junkb = singles.tile([NP, D], b16)
            rtp = singles.tile([NP, 1], f32)
            svec = singles.tile([NP, 1], f32)
            ms = singles.tile([NP, 1], f32)
            outsb = singles.tile([NP, 2], f32)
            nc.vector.scalar_tensor_tensor(
                out=junka[:, :], in0=aux_sb[:, 0:D], scalar=1.0,
                in1=aux_sb[:, D:2 * D], op0=Alu.mult, op1=Alu.mult,
                accum_out=dp[:, :])
            nc.scalar.activation(out=junkb[:, :], in_=aux_sb[:, 0:D],
                                 func=Act.Square, accum_out=sp[:, :])
            nc.scalar.activation(out=rtp[:, :], in_=sp[:, :], func=Act.Sqrt)
            nc.vector.reciprocal(out=rtp[:, :], in_=rtp[:, :])
            nc.vector.tensor_mul(out=svec[:, :], in0=dp[:, :], in1=rtp[:, :])
            nc.vector.tensor_copy(out=outsb[:, 1:2], in_=svec[:, :])
            # ms = margin - s
            nc.scalar.activation(out=ms[:, :], in_=svec[:, :], func=Act.Copy,
                                 bias=float(MARGIN), scale=-1.0)

            # ---- single tail: sim rows -> broadcast -> relu+accum ----
            rt = singles.tile([2 * BL, CH], f32)
            sim = singles.tile([2 * BL, CH], b16)
            junkr = singles.tile([NP, CH], b16)
            nc.scalar.activation(out=rt[:, :], in_=prq[:, :], func=Act.Sqrt)
            nc.vector.reciprocal_approx_fast(out=rt[:, :], in_=rt[:, :])
            nc.vector.tensor_mul(out=sim[:, :], in0=prd[:, :], in1=rt[:, :])
            nc.tensor.matmul(pss[:, :], lhsT=wts_sb[0:2 * BL, WI:WI + NP],
                             rhs=sim[:, :], start=True, stop=True)
            nc.scalar.activation(out=junkr[:, :], in_=pss[:, :],
                                 func=Act.Relu, bias=ms[:, :],
                                 accum_out=outsb[:, 0:1])

            nc.scalar.dma_start(out=out[:, :], in_=outsb[:, :])

    nc.finalize()
    return nc


def _get_nc():
    if "nc" not in _CACHED:
        _CACHED["nc"] = _build_nc()
    return _CACHED["nc"]


def _host_prep(sent, query, pos_idx):
    """Build per-core input maps (bf16, d-major x, packed weights)."""
    import ml_dtypes

    bf16 = ml_dtypes.bfloat16
    sent = np.asarray(sent, dtype=np.float32)
    query = np.asarray(query, dtype=np.float32)
    pos_idx = np.asarray(pos_idx).astype(np.int64)

    qn = np.linalg.norm(query, axis=-1, keepdims=True)
    qhat = (query / np.maximum(qn, 1e-12)).astype(bf16)

    sent16 = sent.astype(bf16)
    # [B, 128, H*L]: xt[b, p, h*L + l] = sent[b, l, 128h + p]
    xt = np.ascontiguousarray(
        sent16.transpose(0, 2, 1)             # [B, D, L]
        .reshape(B, H, 128, L)                # [B, h, p, l]
        .transpose(0, 2, 1, 3)                # [B, p, h, l]
        .reshape(B, 128, H * L))

    in_maps = []
    for core in range(NCORES):
        sl = slice(core * BL, (core + 1) * BL)
        q = qhat[sl]                          # [BL, D]
        wts = np.zeros((128, WTOT), dtype=bf16)
        for b in range(BL):
            for h in range(H):
                wts[:, WD + 12 * (2 * b + h) + 4 + b] = \
                    q[b, 128 * h:128 * h + 128]
            wts[:, WF + 12 * b + 4 + b] = 1.0
            for c in range(C):
                for j in range(P):
                    wts[4 * c + b, WI + 32 * c + 8 * b + j] = 1.0
        aux = np.zeros((2 * BL * P, 2 * D), dtype=bf16)
        for c in range(C):
            for b in range(BL):
                gb = core * BL + b
                r = slice(32 * c + 8 * b, 32 * c + 8 * b + 8)
                aux[r, 0:D] = sent16[gb, pos_idx[gb], :]
                aux[r, D:2 * D] = q[b]
        xc = xt[sl]
        in_maps.append({
            "x0": xc[0],
            "xpr": np.ascontiguousarray(
                np.concatenate([xc[1], xc[2]], axis=1)),
            "x3a": np.ascontiguousarray(xc[3][:, 0:L]),
            "x3b": np.ascontiguousarray(xc[3][:, L:2 * L]),
            "wts": wts,
            "aux": np.ascontiguousarray(aux),
        })
    return in_maps, pos_idx


def _host_finish(results, pos_idx):
    """Combine per-core (G2[c,b,j], s_vec[b,j]) into the scalar loss."""
    g = np.zeros((B, P), dtype=np.float64)
    s = np.zeros((B, P), dtype=np.float64)
    for core, res in enumerate(results):
        o = res["out"].reshape(C, BL, P, 2)
        g[core * BL:(core + 1) * BL] = o[:, :, :, 0].sum(axis=0)
        s[core * BL:(core + 1) * BL] = o[0, :, :, 1]

    loss = 0.0
    total = 0
    for b in range(B):
        _, first = np.unique(pos_idx[b], return_index=True)
        npos = len(first)
        total += npos * (L - npos)
        sb = s[b, first]
        loss += g[b, first].sum()
        loss -= np.maximum(sb[None, :] - sb[:, None] + MARGIN, 0.0).sum()
    return np.float32(loss / total)


def kernel(sent_embeddings, query_embeddings, pos_idx, splits=None, **_):
    import sys
    if "/opt/trn_rl_repo" not in sys.path:
        sys.path.insert(0, "/opt/trn_rl_repo")
    from concourse.bass_utils import run_bass_kernel_spmd

    in_maps, pos_idx = _host_prep(sent_embeddings, query_embeddings, pos_idx)
    nc = _get_nc()
    res = run_bass_kernel_spmd(nc, in_maps, core_ids=list(range(NCORES)))
    _CACHED["last_result"] = res
    return _host_finish(res.results, pos_idx)


if __name__ == "__main__":
    rng = np.random.default_rng(0)
    sent = rng.standard_normal((B, L, D), dtype=np.float32)
    query = rng.standard_normal((B, D), dtype=np.float32)
    pidx = np.stack([rng.choice(L, P, replace=False) for _ in range(B)])
    print(kernel(sent, query, pidx, L))


# revision 10
# speedup vs baseline: 1.0457x; 1.0457x over previous
"""Trainium2 Bass kernel for a contrastive hinge loss.

Problem (B=32 splits, L=1024 candidates/split, P=8 positives/split, D=256):
    e = l2norm(sent), q = l2norm(query)
    sim[b,l] = e[b,l] . q[b]
    loss = sum_{b, p in pos_b, j in neg_b} relu(sim[b,j] - sim[b,p] + margin) / total

Strategy (data-parallel over B across 8 cores, 4 splits per core), v5:
  bf16 inputs (tolerance 2e-2; bf16 keeps the loss within ~2e-5) halve the
  HBM traffic, and x is host-transposed to d-major [d, l] so the dot
  reductions run on the PE.  All dot/ssq results live in ONE [8, 512] psum
  row tile each, partition (4c + b) = l-chunk c of split b, so the whole
  normalization + hinge tail is a single chain:
    prd[4c+b, :] = sum_d q[b,d] x[b,d,l]     (PE, q as 8-col weights)
    prq[4c+b, :] = sum_d x[b,d,l]^2          (squares on DVE for 2 splits and
                                              ACT for 2, PE ones-fold)
    sim = prd * rsqrt(prq)                   (ACT sqrt, DVE approx-recip, mult)
    pss[64, 512] = Ind^T @ sim               (one PE matmul: broadcast to all
                                              (c, b, j) partitions)
    G2[(c,b,j)] = sum_l relu(pss + (margin - s_bj))   (ONE ACT Relu with
                                              per-partition bias + accumulator)
  G[b,j] = G2[0,b,j] + G2[1,b,j] summed on the host.
  s_vec[b,j] comes from host-gathered positive embeddings (aux), computed on
  partitions (c,b,j) directly: STT dot + ACT square-accum + sqrt/recip/mult.
  DMA: the ACT HWDGE ring is ~3-4x faster than the SP ring, so x rides ACT
  (splits 0+1 packed as one tensor for 8 KB descriptor lines, split 3 second)
  plus SWDGE (split 2); wts/aux go on the idle SP ring; out returns on ACT.
  Host finish: dedup positives + subtract pos-pos hinge terms, divide by total.
"""

import numpy as np

B, L, P, D = 32, 1024, 8, 256
NCORES = 8
BL = B // NCORES          # 4 splits per core
H = 2                     # d-halves (256 = 2*128)
C = 2                     # l-chunks (1024 = 2*512 psum-bank limit)
CH = 512
MARGIN = 0.01

# wts column layout (all bf16):
#   dot block for (b,h): 12 cols at WD + 12*(2b+h); q sits at local col 4+b,
#     so slice [4-4c : 12-4c] puts q at col (4c+b) of an 8-wide lhsT.
#   fold block for b: 12 cols at WF + 12*b; ones at local col 4+b, same trick.
#   ind block: 64 cols at WI; wts[4c+b, WI + 32c + 8b + j] = 1.
WD = 0
WF = WD + 12 * BL * H     # 96
WI = WF + 12 * BL         # 144
WTOT = WI + 2 * BL * P    # 208

_CACHED = {}


def _build_nc():
    import concourse.bass as bass
    import concourse.mybir as mybir
    import concourse.tile as tile
    from concourse import bacc

    f32 = mybir.dt.float32
    b16 = mybir.dt.bfloat16
    Alu = mybir.AluOpType
    Act = mybir.ActivationFunctionType

    nc = bacc.Bacc("TRN2")
    # x[b][p, h*1024 + l] = sent_T[b, 128h + p, l]  (host-transposed, bf16).
    # Splits 0+1 ride one tensor so each partition line is 8 KB contiguous --
    # the HWDGE rings are descriptor-rate-limited, so bigger lines = more B/s.
    xpr = nc.dram_tensor("xpr", [128, 2 * H * L], b16, kind="ExternalInput")
    x2 = nc.dram_tensor("x2", [128, H * L], b16, kind="ExternalInput")
    x3 = nc.dram_tensor("x3", [128, H * L], b16, kind="ExternalInput")
    wts = nc.dram_tensor("wts", [128, WTOT], b16, kind="ExternalInput")
    # aux[32c + 8b + j, 0:256] = sent[b, pos_idx[b,j], :]; [., 256:512] = qhat[b]
    aux = nc.dram_tensor("aux", [2 * BL * P, 2 * D], b16, kind="ExternalInput")
    # out[32c + 8b + j] = (G2[c,b,j], s_vec[b,j])
    out = nc.dram_tensor("out", [2 * BL * P, 2], f32, kind="ExternalOutput")

    NP = 2 * BL * P       # 64 (c,b,j) partitions

    with tile.TileContext(nc) as tc:
        with (
            tc.tile_pool(name="singles", bufs=1) as singles,
            tc.tile_pool(name="xpool", bufs=4) as xpool,
            tc.tile_pool(name="sqpool", bufs=8) as sqpool,
            tc.tile_pool(name="pp", bufs=1, space="PSUM") as pp,
        ):
            # ---- aux rides FIRST on the fast ACT ring (tiny) so the
            # s_vec chain never blocks the DVE queue; then the x bulk:
            # splits 0+1 packed for 8 KB descriptor lines, split 3 second on
            # ACT, split 2 on SWDGE; wts on the idle SP ring. ----
            aux_sb = singles.tile([NP, 2 * D], b16)
            nc.scalar.dma_start(out=aux_sb[:, :], in_=aux[:, :])
            xpr_sb = xpool.tile([128, 2 * H * L], b16, tag="xpr")
            nc.scalar.dma_start(out=xpr_sb[:, :], in_=xpr[:, :])
            xt3 = xpool.tile([128, H * L], b16, tag="xt3")
            nc.scalar.dma_start(out=xt3[:, :], in_=x3[:, :])
            xt2 = xpool.tile([128, H * L], b16, tag="xt2")
            nc.gpsimd.dma_start(out=xt2[:, :], in_=x2[:, :])
            wts_sb = singles.tile([128, WTOT], b16)
            nc.sync.dma_start(out=wts_sb[:, :], in_=wts[:, :])
            # per (b, h): the [128, L] block of x for that split/d-half
            xh = [[xpr_sb[:, 0:L], xpr_sb[:, L:2 * L]],
                  [xpr_sb[:, 2 * L:3 * L], xpr_sb[:, 3 * L:4 * L]],
                  [xt2[:, 0:L], xt2[:, L:2 * L]],
                  [xt3[:, 0:L], xt3[:, L:2 * L]]]

            # ---- ACT table warm (sqrt_and_others: Sqrt/Square/Relu/Copy) ----
            warm = singles.tile([1, 1], f32)
            nc.vector.memset(warm[:, :], 1.0)
            nc.scalar.activation(out=warm[0:1, :], in_=warm[0:1, :],
                                 func=Act.Sqrt)

            # ---- PE warm-up: keep the HAM busy through the DMA window so
            # real matmuls run at 2.4 GHz instead of 1.2.
            junkw = singles.tile([128, 516], b16)
            nc.vector.memset(junkw[:, :], 0.125)
            pjunk = pp.tile([4, CH], f32, name="pjunk", tag="pjunk")
            for w in range(10):
                nc.tensor.matmul(pjunk[:, :], lhsT=junkw[:, 0:4],
                                 rhs=junkw[:, 4:516], start=True, stop=True)

            # ---- heavy pass: dot rows prd, ssq rows prq at (4c+b) ----
            prd = pp.tile([2 * BL, CH], f32, name="prd", tag="prd")
            prq = pp.tile([2 * BL, CH], f32, name="prq", tag="prq")
            pss = pp.tile([NP, CH], f32, name="pss", tag="pss")

            nd = 0
            nq = 0
            NTOT = BL * H * C     # 16 MMs in each of the two groups

            # squares run per (split, d-half) so the last split's square is
            # short; ACT takes two halves so DVE's chain stays under ~4.5us
            SQ_ACT = {(0, 1), (1, 1), (3, 1)}
            for b in [0, 1, 3, 2]:
                for h in range(H):
                    xbh = xh[b][h]
                    blk = WD + 12 * (2 * b + h)
                    for c in range(C):
                        nc.tensor.matmul(
                            prd[:, :],
                            lhsT=wts_sb[:, blk + 4 - 4 * c:blk + 12 - 4 * c],
                            rhs=xbh[:, c * CH:c * CH + CH],
                            start=(nd == 0),
                            stop=(nd == NTOT - 1),
                        )
                        nd += 1
                    sq = sqpool.tile([128, L], b16, tag="sq")
                    if (b, h) in SQ_ACT:
                        nc.scalar.activation(out=sq[:, :], in_=xbh[:, :],
                                             func=Act.Square)
                    else:
                        nc.vector.tensor_mul(out=sq[:, :], in0=xbh[:, :],
                                             in1=xbh[:, :])
                    fblk = WF + 12 * b
                    for c in range(C):
                        nc.tensor.matmul(
                            prq[:, :],
                            lhsT=wts_sb[:, fblk + 4 - 4 * c:fblk + 12 - 4 * c],
                            rhs=sq[:, c * CH:c * CH + CH],
                            start=(nq == 0),
                            stop=(nq == NTOT - 1),
                        )
                        nq += 1

            # ---- s_vec on (c,b,j) partitions (emitted late: only needed as
            # the relu bias; aux also arrives late on the SP ring) ----
            dp = singles.tile([NP, 1], f32)
            sp = singles.tile([NP, 1], f32)
            junka = singles.tile([NP, D], b16)
            junkb = singles.tile([NP, D], b16)
            rtp = singles.tile([NP, 1], f32)
            svec = singles.tile([NP, 1], f32)
            ms = singles.tile([NP, 1], f32)
            outsb = singles.tile([NP, 2], f32)
            nc.vector.scalar_tensor_tensor(
                out=junka[:, :], in0=aux_sb[:, 0:D], scalar=1.0,
                in1=aux_sb[:, D:2 * D], op0=Alu.mult, op1=Alu.mult,
                accum_out=dp[:, :])
            nc.scalar.activation(out=junkb[:, :], in_=aux_sb[:, 0:D],
                                 func=Act.Square, accum_out=sp[:, :])
            nc.scalar.activation(out=rtp[:, :], in_=sp[:, :], func=Act.Sqrt)
            nc.vector.reciprocal(out=rtp[:, :], in_=rtp[:, :])
            nc.vector.tensor_mul(out=svec[:, :], in0=dp[:, :], in1=rtp[:, :])
            nc.vector.tensor_copy(out=outsb[:, 1:2], in_=svec[:, :])
            # ms = margin - s
            nc.scalar.activation(out=ms[:, :], in_=svec[:, :], func=Act.Copy,
                                 bias=float(MARGIN), scale=-1.0)

            # ---- single tail: sim rows -> broadcast -> relu+accum ----
            rt = singles.tile([2 * BL, CH], f32)
            sim = singles.tile([2 * BL, CH], b16)
            junkr = singles.tile([NP, CH], b16)
            nc.scalar.activation(out=rt[:, :], in_=prq[:, :], func=Act.Sqrt)
            nc.vector.reciprocal_approx_fast(out=rt[:, :], in_=rt[:, :])
            nc.vector.tensor_mul(out=sim[:, :], in0=prd[:, :], in1=rt[:, :])
            nc.tensor.matmul(pss[:, :], lhsT=wts_sb[0:2 * BL, WI:WI + NP],
                             rhs=sim[:, :], start=True, stop=True)
            nc.scalar.activation(out=junkr[:, :], in_=pss[:, :],
                                 func=Act.Relu, bias=ms[:, :],
                                 accum_out=outsb[:, 0:1])

            nc.scalar.dma_start(out=out[:, :], in_=outsb[:, :])

    nc.finalize()
    return nc


def _get_nc():
    if "nc" not in _CACHED:
        _CACHED["nc"] = _build_nc()
    return _CACHED["nc"]


def _host_prep(sent, query, pos_idx):
    """Build per-core input maps (bf16, d-major x, packed weights)."""
    import ml_dtypes

    bf16 = ml_dtypes.bfloat16
    sent = np.asarray(sent, dtype=np.float32)
    query = np.asarray(query, dtype=np.float32)
    pos_idx = np.asarray(pos_idx).astype(np.int64)

    qn = np.linalg.norm(query, axis=-1, keepdims=True)
    qhat = (query / np.maximum(qn, 1e-12)).astype(bf16)

    sent16 = sent.astype(bf16)
    # [B, 128, H*L]: xt[b, p, h*L + l] = sent[b, l, 128h + p]
    xt = np.ascontiguousarray(
        sent16.transpose(0, 2, 1)             # [B, D, L]
        .reshape(B, H, 128, L)                # [B, h, p, l]
        .transpose(0, 2, 1, 3)                # [B, p, h, l]
        .reshape(B, 128, H * L))

    in_maps = []
    for core in range(NCORES):
        sl = slice(core * BL, (core + 1) * BL)
        q = qhat[sl]                          # [BL, D]
        wts = np.zeros((128, WTOT), dtype=bf16)
        for b in range(BL):
            for h in range(H):
                wts[:, WD + 12 * (2 * b + h) + 4 + b] = \
                    q[b, 128 * h:128 * h + 128]
            wts[:, WF + 12 * b + 4 + b] = 1.0
            for c in range(C):
                for j in range(P):
                    wts[4 * c + b, WI + 32 * c + 8 * b + j] = 1.0
        aux = np.zeros((2 * BL * P, 2 * D), dtype=bf16)
        for c in range(C):
            for b in range(BL):
                gb = core * BL + b
                r = slice(32 * c + 8 * b, 32 * c + 8 * b + 8)
                aux[r, 0:D] = sent16[gb, pos_idx[gb], :]
                aux[r, D:2 * D] = q[b]
        xc = xt[sl]
        in_maps.append({
            "xpr": np.ascontiguousarray(
                np.concatenate([xc[0], xc[1]], axis=1)),
            "x2": xc[2],
            "x3": xc[3],
            "wts": wts,
            "aux": np.ascontiguousarray(aux),
        })
    return in_maps, pos_idx


def _host_finish(results, pos_idx):
    """Combine per-core (G2[c,b,j], s_vec[b,j]) into the scalar loss."""
    g = np.zeros((B, P), dtype=np.float64)
    s = np.zeros((B, P), dtype=np.float64)
    for core, res in enumerate(results):
        o = res["out"].reshape(C, BL, P, 2)
        g[core * BL:(core + 1) * BL] = o[:, :, :, 0].sum(axis=0)
        s[core * BL:(core + 1) * BL] = o[0, :, :, 1]

    loss = 0.0
    total = 0
    for b in range(B):
        _, first = np.unique(pos_idx[b], return_index=True)
        npos = len(first)
        total += npos * (L - npos)
        sb = s[b, first]
        loss += g[b, first].sum()
        loss -= np.maximum(sb[None, :] - sb[:, None] + MARGIN, 0.0).sum()
    return np.float32(loss / total)


def kernel(sent_embeddings, query_embeddings, pos_idx, splits=None, **_):
    import sys
    if "/opt/trn_rl_repo" not in sys.path:
        sys.path.insert(0, "/opt/trn_rl_repo")
    from concourse.bass_utils import run_bass_kernel_spmd

    in_maps, pos_idx = _host_prep(sent_embeddings, query_embeddings, pos_idx)
    nc = _get_nc()
    res = run_bass_kernel_spmd(nc, in_maps, core_ids=list(range(NCORES)))
    _CACHED["last_result"] = res
    return _host_finish(res.results, pos_idx)


if __name__ == "__main__":
    rng = np.random.default_rng(0)
    sent = rng.standard_normal((B, L, D), dtype=np.float32)
    query = rng.standard_normal((B, D), dtype=np.float32)
    pidx = np.stack([rng.choice(L, P, replace=False) for _ in range(B)])
    print(kernel(sent, query, pidx, L))


# revision 11
# speedup vs baseline: 1.0870x; 1.0395x over previous
"""Trainium2 Bass kernel for a contrastive hinge loss.

Problem (B=32 splits, L=1024 candidates/split, P=8 positives/split, D=256):
    e = l2norm(sent), q = l2norm(query)
    sim[b,l] = e[b,l] . q[b]
    loss = sum_{b, p in pos_b, j in neg_b} relu(sim[b,j] - sim[b,p] + margin) / total

Strategy (data-parallel over B across 8 cores, 4 splits per core), v5:
  bf16 inputs (tolerance 2e-2; bf16 keeps the loss within ~2e-5) halve the
  HBM traffic, and x is host-transposed to d-major [d, l] so the dot
  reductions run on the PE.  All dot/ssq results live in ONE [8, 512] psum
  row tile each, partition (4c + b) = l-chunk c of split b, so the whole
  normalization + hinge tail is a single chain:
    prd[4c+b, :] = sum_d q[b,d] x[b,d,l]     (PE, q as 8-col weights)
    prq[4c+b, :] = sum_d x[b,d,l]^2          (squares on DVE for 2 splits and
                                              ACT for 2, PE ones-fold)
    sim = prd * rsqrt(prq)                   (ACT sqrt, DVE approx-recip, mult)
    pss[64, 512] = Ind^T @ sim               (one PE matmul: broadcast to all
                                              (c, b, j) partitions)
    G2[(c,b,j)] = sum_l relu(pss + (margin - s_bj))   (ONE ACT Relu with
                                              per-partition bias + accumulator)
  G[b,j] = G2[0,b,j] + G2[1,b,j] summed on the host.
  s_vec[b,j] comes from host-gathered positive embeddings (aux), computed on
  partitions (c,b,j) directly: STT dot + ACT square-accum + sqrt/recip/mult.
  DMA: the ACT HWDGE ring is ~3-4x faster than the SP ring, so x rides ACT
  (splits 0+1 packed as one tensor for 8 KB descriptor lines, split 3 second)
  plus SWDGE (split 2); wts/aux go on the idle SP ring; out returns on ACT.
  Host finish: dedup positives + subtract pos-pos hinge terms, divide by total.
"""

import numpy as np

B, L, P, D = 32, 1024, 8, 256
NCORES = 8
BL = B // NCORES          # 4 splits per core
H = 2                     # d-halves (256 = 2*128)
C = 2                     # l-chunks (1024 = 2*512 psum-bank limit)
CH = 512
MARGIN = 0.01

# wts column layout (all bf16):
#   dot block for (b,h): 12 cols at WD + 12*(2b+h); q sits at local col 4+b,
#     so slice [4-4c : 12-4c] puts q at col (4c+b) of an 8-wide lhsT.
#   fold block for b: 12 cols at WF + 12*b; ones at local col 4+b, same trick.
#   ind block: 64 cols at WI; wts[4c+b, WI + 32c + 8b + j] = 1.
WD = 0
WF = WD + 12 * BL * H     # 96
WI = WF + 12 * BL         # 144
WTOT = WI + 2 * BL * P    # 208

_CACHED = {}


def _build_nc():
    import concourse.bass as bass
    import concourse.mybir as mybir
    import concourse.tile as tile
    from concourse import bacc

    f32 = mybir.dt.float32
    b16 = mybir.dt.bfloat16
    Alu = mybir.AluOpType
    Act = mybir.ActivationFunctionType

    nc = bacc.Bacc("TRN2")
    # x[b][p, h*1024 + l] = sent_T[b, 128h + p, l]  (host-transposed, bf16).
    # Splits 0+1 ride one tensor so each partition line is 8 KB contiguous --
    # the HWDGE rings are descriptor-rate-limited, so bigger lines = more B/s.
    xpr = nc.dram_tensor("xpr", [128, 2 * H * L], b16, kind="ExternalInput")
    x2 = nc.dram_tensor("x2", [128, H * L], b16, kind="ExternalInput")
    x3 = nc.dram_tensor("x3", [128, H * L], b16, kind="ExternalInput")
    wts = nc.dram_tensor("wts", [128, WTOT], b16, kind="ExternalInput")
    # aux[32c + 8b + j, 0:256] = sent[b, pos_idx[b,j], :]; [., 256:512] = qhat[b]
    aux = nc.dram_tensor("aux", [2 * BL * P, 2 * D], b16, kind="ExternalInput")
    # out[32c + 8b + j] = (G2[c,b,j], s_vec[b,j])
    out = nc.dram_tensor("out", [2 * BL * P, 2], f32, kind="ExternalOutput")

    NP = 2 * BL * P       # 64 (c,b,j) partitions

    with tile.TileContext(nc) as tc:
        with (
            tc.tile_pool(name="singles", bufs=1) as singles,
            tc.tile_pool(name="xpool", bufs=4) as xpool,
            tc.tile_pool(name="sqpool", bufs=8) as sqpool,
            tc.tile_pool(name="pp", bufs=1, space="PSUM") as pp,
        ):
            # ---- x loads first on their rings ----
            xpr_sb = xpool.tile([128, 2 * H * L], b16, tag="xpr")
            nc.scalar.dma_start(out=xpr_sb[:, :], in_=xpr[:, :])
            xt3 = xpool.tile([128, H * L], b16, tag="xt3")
            nc.scalar.dma_start(out=xt3[:, :], in_=x3[:, :])
            xt2 = xpool.tile([128, H * L], b16, tag="xt2")
            nc.gpsimd.dma_start(out=xt2[:, :], in_=x2[:, :])
            xts = [xpr_sb[:, 0:H * L], xpr_sb[:, H * L:2 * H * L], xt2, xt3]

            # small loads on the otherwise idle SP ring
            wts_sb = singles.tile([128, WTOT], b16)
            nc.sync.dma_start(out=wts_sb[:, :], in_=wts[:, :])
            aux_sb = singles.tile([NP, 2 * D], b16)
            nc.sync.dma_start(out=aux_sb[:, :], in_=aux[:, :])

            # ---- ACT table warm (sqrt_and_others: Sqrt/Square/Relu/Copy) ----
            warm = singles.tile([1, 1], f32)
            nc.vector.memset(warm[:, :], 1.0)
            nc.scalar.activation(out=warm[0:1, :], in_=warm[0:1, :],
                                 func=Act.Sqrt)

            # ---- PE warm-up: keep the HAM busy through the DMA window so
            # real matmuls run at 2.4 GHz instead of 1.2.
            junkw = singles.tile([128, 516], b16)
            nc.vector.memset(junkw[:, :], 0.125)
            pjunk = pp.tile([4, CH], f32, name="pjunk", tag="pjunk")
            for w in range(10):
                nc.tensor.matmul(pjunk[:, :], lhsT=junkw[:, 0:4],
                                 rhs=junkw[:, 4:516], start=True, stop=True)

            # ---- heavy pass: dot rows prd, ssq rows prq at (4c+b) ----
            prd = pp.tile([2 * BL, CH], f32, name="prd", tag="prd")
            prq = pp.tile([2 * BL, CH], f32, name="prq", tag="prq")
            pss = pp.tile([NP, CH], f32, name="pss", tag="pss")

            nd = 0
            nq = 0
            NTOT = BL * H * C     # 16 MMs in each of the two groups

            for b in [0, 1, 3, 2]:
                xt = xts[b]
                # dot: prd[4c+b, :] += qhat[b,h] . x[b,h,chunk c]
                for h in range(H):
                    blk = WD + 12 * (2 * b + h)
                    for c in range(C):
                        nc.tensor.matmul(
                            prd[:, :],
                            lhsT=wts_sb[:, blk + 4 - 4 * c:blk + 12 - 4 * c],
                            rhs=xt[:, h * L + c * CH:h * L + c * CH + CH],
                            start=(nd == 0),
                            stop=(nd == NTOT - 1),
                        )
                        nd += 1
                # squares: one big bf16 op per split, alternating DVE/ACT so
                # the four don't serialize on one engine
                sq = sqpool.tile([128, H * L], b16, tag="sq")
                if b in (0, 2):
                    nc.vector.tensor_mul(out=sq[:, :], in0=xt[:, :],
                                         in1=xt[:, :])
                else:
                    nc.scalar.activation(out=sq[:, :], in_=xt[:, :],
                                         func=Act.Square)
                fblk = WF + 12 * b
                for c in range(C):
                    for h in range(H):
                        nc.tensor.matmul(
                            prq[:, :],
                            lhsT=wts_sb[:, fblk + 4 - 4 * c:fblk + 12 - 4 * c],
                            rhs=sq[:, h * L + c * CH:h * L + c * CH + CH],
                            start=(nq == 0),
                            stop=(nq == NTOT - 1),
                        )
                        nq += 1

            # ---- s_vec on (c,b,j) partitions (emitted late: only needed as
            # the relu bias; aux also arrives late on the SP ring) ----
            dp = singles.tile([NP, 1], f32)
            sp = singles.tile([NP, 1], f32)
            junka = singles.tile([NP, D], b16)
            junkb = singles.tile([NP, D], b16)
            rtp = singles.tile([NP, 1], f32)
            svec = singles.tile([NP, 1], f32)
            ms = singles.tile([NP, 1], f32)
            outsb = singles.tile([NP, 2], f32)
            nc.vector.scalar_tensor_tensor(
                out=junka[:, :], in0=aux_sb[:, 0:D], scalar=1.0,
                in1=aux_sb[:, D:2 * D], op0=Alu.mult, op1=Alu.mult,
                accum_out=dp[:, :])
            nc.scalar.activation(out=junkb[:, :], in_=aux_sb[:, 0:D],
                                 func=Act.Square, accum_out=sp[:, :])
            nc.scalar.activation(out=rtp[:, :], in_=sp[:, :], func=Act.Sqrt)
            nc.vector.reciprocal(out=rtp[:, :], in_=rtp[:, :])
            nc.vector.tensor_mul(out=svec[:, :], in0=dp[:, :], in1=rtp[:, :])
            nc.vector.tensor_copy(out=outsb[:, 1:2], in_=svec[:, :])
            # ms = margin - s
            nc.scalar.activation(out=ms[:, :], in_=svec[:, :], func=Act.Copy,
                                 bias=float(MARGIN), scale=-1.0)

            # ---- single tail: sim rows -> broadcast -> relu+accum ----
            rt = singles.tile([2 * BL, CH], f32)
            sim = singles.tile([2 * BL, CH], b16)
            junkr = singles.tile([NP, CH], b16)
            nc.scalar.activation(out=rt[:, :], in_=prq[:, :], func=Act.Sqrt)
            nc.vector.reciprocal_approx_fast(out=rt[:, :], in_=rt[:, :])
            nc.vector.tensor_mul(out=sim[:, :], in0=prd[:, :], in1=rt[:, :])
            nc.tensor.matmul(pss[:, :], lhsT=wts_sb[0:2 * BL, WI:WI + NP],
                             rhs=sim[:, :], start=True, stop=True)
            nc.scalar.activation(out=junkr[:, :], in_=pss[:, :],
                                 func=Act.Relu, bias=ms[:, :],
                                 accum_out=outsb[:, 0:1])

            nc.scalar.dma_start(out=out[:, :], in_=outsb[:, :])

    nc.finalize()
    return nc


def _get_nc():
    if "nc" not in _CACHED:
        _CACHED["nc"] = _build_nc()
    return _CACHED["nc"]


def _host_prep(sent, query, pos_idx):
    """Build per-core input maps (bf16, d-major x, packed weights)."""
    import ml_dtypes

    bf16 = ml_dtypes.bfloat16
    sent = np.asarray(sent, dtype=np.float32)
    query = np.asarray(query, dtype=np.float32)
    pos_idx = np.asarray(pos_idx).astype(np.int64)

    qn = np.linalg.norm(query, axis=-1, keepdims=True)
    qhat = (query / np.maximum(qn, 1e-12)).astype(bf16)

    sent16 = sent.astype(bf16)
    # [B, 128, H*L]: xt[b, p, h*L + l] = sent[b, l, 128h + p]
    xt = np.ascontiguousarray(
        sent16.transpose(0, 2, 1)             # [B, D, L]
        .reshape(B, H, 128, L)                # [B, h, p, l]
        .transpose(0, 2, 1, 3)                # [B, p, h, l]
        .reshape(B, 128, H * L))

    in_maps = []
    for core in range(NCORES):
        sl = slice(core * BL, (core + 1) * BL)
        q = qhat[sl]                          # [BL, D]
        wts = np.zeros((128, WTOT), dtype=bf16)
        for b in range(BL):
            for h in range(H):
                wts[:, WD + 12 * (2 * b + h) + 4 + b] = \
                    q[b, 128 * h:128 * h + 128]
            wts[:, WF + 12 * b + 4 + b] = 1.0
            for c in range(C):
                for j in range(P):
                    wts[4 * c + b, WI + 32 * c + 8 * b + j] = 1.0
        aux = np.zeros((2 * BL * P, 2 * D), dtype=bf16)
        for c in range(C):
            for b in range(BL):
                gb = core * BL + b
                r = slice(32 * c + 8 * b, 32 * c + 8 * b + 8)
                aux[r, 0:D] = sent16[gb, pos_idx[gb], :]
                aux[r, D:2 * D] = q[b]
        xc = xt[sl]
        in_maps.append({
            "xpr": np.ascontiguousarray(
                np.concatenate([xc[0], xc[1]], axis=1)),
            "x2": xc[2],
            "x3": xc[3],
            "wts": wts,
            "aux": np.ascontiguousarray(aux),
        })
    return in_maps, pos_idx


def _host_finish(results, pos_idx):
    """Combine per-core (G2[c,b,j], s_vec[b,j]) into the scalar loss."""
    g = np.zeros((B, P), dtype=np.float64)
    s = np.zeros((B, P), dtype=np.float64)
    for core, res in enumerate(results):
        o = res["out"].reshape(C, BL, P, 2)
        g[core * BL:(core + 1) * BL] = o[:, :, :, 0].sum(axis=0)
        s[core * BL:(core + 1) * BL] = o[0, :, :, 1]

    loss = 0.0
    total = 0
    for b in range(B):
        _, first = np.unique(pos_idx[b], return_index=True)
        npos = len(first)
        total += npos * (L - npos)
        sb = s[b, first]
        loss += g[b, first].sum()
        loss -= np.maximum(sb[None, :] - sb[:, None] + MARGIN, 0.0).sum()
    return np.float32(loss / total)


def kernel(sent_embeddings, query_embeddings, pos_idx, splits=None, **_):
    import sys
    if "/opt/trn_rl_repo" not in sys.path:
        sys.path.insert(0, "/opt/trn_rl_repo")
    from concourse.bass_utils import run_bass_kernel_spmd

    in_maps, pos_idx = _host_prep(sent_embeddings, query_embeddings, pos_idx)
    nc = _get_nc()
    res = run_bass_kernel_spmd(nc, in_maps, core_ids=list(range(NCORES)))
    _CACHED["last_result"] = res
    return _host_finish(res.results, pos_idx)


if __name__ == "__main__":
    rng = np.random.default_rng(0)
    sent = rng.standard_normal((B, L, D), dtype=np.float32)
    query = rng.standard_normal((B, D), dtype=np.float32)
    pidx = np.stack([rng.choice(L, P, replace=False) for _ in range(B)])
    print(kernel(sent, query, pidx, L))
